# revision 35
# baseline (speedup 1.0000x reference)
"""Masked dot-product attention on 8 TRN2 NeuronCores (Bass/Tile).

Strategy (fixed problem shape B=16, NQ=NK=2048, D=DV=128):

* Work per batch is proportional to ceil(L_b/128) k-tiles.  Every core runs
  the same SPMD program of S slots with fixed tile-extents [e_0..e_S-1]; the
  host assigns each (core, slot) one contiguous (batch, k-range) segment at
  kernel-call time (recomputed from the actual valid_lens).  Segments of one
  batch may land on different cores/slots; the host sums the partials.

* Device math per k-tile t (128 keys), per q-half (1024 queries):
    S^T[k,q]   = K_tile @ Q^T          (bf16 matmuls, full PE rate; fp32r
                 measured ~1.6 cyc/col on HW vs bf16's 1.0)
    P^T[k,q]   = exp(S^T/sqrt(D) + bias_k)   (one ScalarE op: scale+bias+exp
                 fused; bias is -1e6 for masked/padded keys so P underflows
                 to exactly 0 -> no max-subtraction, no separate masking)
    O^T[v,q]  += V_tile^T-contraction matmul (bf16)     [PSUM accumulate]
    denominator: P tiles are quad-summed on DVE (bf16 2x) and the quad
    sums are DMA'd to HBM; the HOST does the k-partition reduction --
    no ones-matmuls on PE, no [1,1024] PSUM->SBUF copies on DVE.
  Everything stays in the transposed (k-on-partitions) orientation: no
  on-device transposes; the host pre-transposes Q and K once.

* Perf structure (from NTFF trace analysis):
  - ScalarE exp (~1.0-1.1us per tile-half) is the floor; everything else
    is arranged to keep ScalarE 100% busy: spool bufs=3 lets PE run S one
    tile ahead, and the last PV + output copy of each slot-half are
    ISSUED after the next half's S+exp (cross-boundary software pipeline)
    so ScalarE never waits out a boundary and the output copy never
    blocks younger DVE work at the strict-FIFO queue head.
  - PE HAM clock gate: ~3.4us of back-to-back warmup matmuls (overlapping
    the initial input DMA) flips the PE to 2.4GHz before real work starts.
  - DMA queues: inputs (bias/kt/qt/v) ride the sync queue, outputs ride the
    gpsimd queue.  DGE rings are in-order, so mixing them lets next-slot
    input prefetch get stuck behind current-slot output drains.  The very
    last output chunks go on the (by then idle) sync queue.

* The host divides the accumulated numerator by the denominator and
  transposes back.
"""

import math

import ml_dtypes
import numpy as np

import concourse.bass as bass  # noqa: F401  (bass types used via tile/bacc)
import concourse.mybir as mybir
import concourse.tile as tile
from concourse import bacc
from concourse.bass_utils import run_bass_kernel_spmd

B, NQ, NK, D, DV = 16, 2048, 2048, 128, 128
NCORES = 8
KT = 128  # keys per k-tile (partition dim)
QH = 1024  # queries per q-half (PSUM sizing)
NEG = np.float32(-1.0e6)
SCALE = 1.0 / math.sqrt(D)

F32 = mybir.dt.float32
BF16 = mybir.dt.bfloat16

_PROGRAM_CACHE: dict[tuple, object] = {}
LAST_RESULT = None  # BassKernelResults of the most recent run (for test.py)


# ---------------------------------------------------------------- scheduling
def _greedy_fill(sizes, extents, n_cores):
    """Slot-major greedy: biggest-remaining batch gets each segment."""
    rem = list(sizes)
    nxt = [0] * len(sizes)
    assign = [[None] * len(extents) for _ in range(n_cores)]
    for p, e in enumerate(extents):
        for c in range(n_cores):
            b = max(range(len(rem)), key=lambda i: rem[i])
            if rem[b] <= 0:
                continue
            seg = min(rem[b], e)
            assign[c][p] = (b, nxt[b], seg)
            nxt[b] += seg
            rem[b] -= seg
    if any(r > 0 for r in rem):
        return None
    return assign


def _exact_fill(sizes, extents, n_cores, budget=4000):
    """Cut batches into pieces and place each piece in its own (core, slot)
    segment (capacity = that slot's extent).  Greedy fast path, then DFS
    over cap-type choices with a node budget; returns assign[core][pos] =
    (batch, tile_start, n) or None if no assignment found."""
    g = _greedy_fill(sizes, extents, n_cores)
    if g is not None:
        return g
    caps = sorted(set(extents), reverse=True)
    cap_cnt = {c: 0 for c in caps}
    for e in extents:
        cap_cnt[e] += n_cores
    bud = [budget]
    seen = set()

    def dfs(rem, cnt):
        if bud[0] <= 0:
            return None
        bud[0] -= 1
        live = sorted(((r, b) for b, r in enumerate(rem) if r > 0), reverse=True)
        if not live:
            return []
        key = (tuple(r for r, _ in live), tuple(cnt[c] for c in caps))
        if key in seen:
            return None
        if sum(c * cnt[c] for c in caps) < sum(r for r, _ in live):
            seen.add(key)
            return None
        if sum(cnt[c] for c in caps) < len(live):
            seen.add(key)
            return None
        r, b = live[0]
        for c in caps:
            if cnt[c] == 0:
                continue
            piece = min(r, c)
            rem[b] -= piece
            cnt[c] -= 1
            sub = dfs(rem, cnt)
            rem[b] += piece
            cnt[c] += 1
            if sub is not None:
                return [(b, piece, c)] + sub
        seen.add(key)
        return None

    pieces = dfs(list(sizes), dict(cap_cnt))
    if pieces is None:
        return None
    # materialize: per cap-type, queue of (slot_pos, core) pairs
    slots_by_cap = {c: [] for c in caps}
    for p, e in enumerate(extents):
        for core in range(n_cores):
            slots_by_cap[e].append((p, core))
    assign = [[None] * len(extents) for _ in range(n_cores)]
    nxt = [0] * len(sizes)
    for b, piece, c in pieces:
        p, core = slots_by_cap[c].pop()
        assign[core][p] = (b, nxt[b], piece)
        nxt[b] += piece
    return assign


def _partitions(tot, nparts, maxpart):
    """Descending tuples of nparts positive ints summing to tot."""
    if nparts == 1:
        if tot <= maxpart:
            yield (tot,)
        return
    lo = (tot + nparts - 1) // nparts
    for e in range(min(maxpart, tot - nparts + 1), lo - 1, -1):
        for rest in _partitions(tot - e, nparts - 1, e):
            yield (e,) + rest


def _schedule(sizes, n_cores=NCORES):
    """Pick slot extents minimizing executed tiles per core, then slots."""
    total = sum(sizes)
    cap = (total + n_cores - 1) // n_cores
    mx = min(16, max(sizes))
    for tot in range(cap, cap + 8):
        for nslots in range(1, 5):
            cands = sorted(
                _partitions(tot, nslots, min(mx, tot)), key=lambda t: -min(t)
            )
            for extents in cands:
                a = _exact_fill(sizes, extents, n_cores)
                if a is not None:
                    return list(extents), a
    raise AssertionError("scheduler failed")


# ------------------------------------------------------------ device program
def _build(extents):
    nc = bacc.Bacc()
    # fewer DMA rings -> fewer per-ring semaphores to reset in the NEFF
    # epilogue (each reset is its own ~115ns instruction)
    for q in nc.m.queues:
        q.num_queues = 8
    S = len(extents)
    qt_d, kt_d, v_d, b_d, o_d, d_d = [], [], [], [], [], []
    for s, e in enumerate(extents):
        qt_d.append(nc.dram_tensor(f"qt{s}", [D, NQ], BF16, kind="ExternalInput"))
        # K^T d-major [D, e*KT]: one DMA, per-partition contiguous lines
        kt_d.append(nc.dram_tensor(f"kt{s}", [D, e * KT], BF16, kind="ExternalInput"))
        # V pre-blocked on host to SBUF image [k-within-tile, tile*DV]
        v_d.append(nc.dram_tensor(f"v{s}", [KT, e * DV], BF16, kind="ExternalInput"))
        b_d.append(nc.dram_tensor(f"b{s}", [KT, e], F32, kind="ExternalInput"))
        o_d.append(nc.dram_tensor(f"o{s}", [DV, NQ], BF16, kind="ExternalOutput"))
        # P quad-sums (bf16): host reduces over k-partitions for the
        # denominator -> no ones-matmuls, no [1,1024] PSUM->SBUF copies
        g = (e + 3) // 4
        d_d.append(
            nc.dram_tensor(f"pq{s}", [2 * g, KT, QH], BF16, kind="ExternalOutput")
        )

    with tile.TileContext(nc) as tc:
        with (
            tc.tile_pool(name="const", bufs=1) as cpool,
            tc.tile_pool(name="qt", bufs=3) as qpool,
            tc.tile_pool(name="kt", bufs=3) as kpool,
            tc.tile_pool(name="v", bufs=3) as vpool,
            tc.tile_pool(name="bias", bufs=3) as bpool,
            tc.tile_pool(name="p", bufs=12) as ppool,
            tc.tile_pool(name="ps", bufs=3) as pspool,
            tc.tile_pool(name="pq", bufs=3) as pqpool,
            tc.tile_pool(name="osb", bufs=2) as opool_sb,
            tc.tile_pool(name="spsum", bufs=3, space="PSUM") as spool,
            tc.tile_pool(name="opsum", bufs=1, space="PSUM") as opool,
        ):
            ones = cpool.tile([KT, 1], BF16)
            nc.vector.memset(ones[:], 1.0)
            # warmup: ~4.3us of back-to-back 512-col matmuls during input DMA
            # fills a HAM activity window -> PE at 2.4GHz when real S starts.
            # Also triggers the exp ACT-table load.
            wsrc = cpool.tile([KT, 512], BF16)
            nc.vector.memset(wsrc[:], 0.0)
            wpt = ppool.tile([KT, QH], BF16, tag="pt")
            nc.scalar.activation(
                wpt[:, :128], wsrc[:, :128], mybir.ActivationFunctionType.Exp
            )
            wps = opool.tile([DV, QH], F32, tag="opsum")
            # 5 is enough: the real S matmuls continue the HAM activity
            # window back-to-back, so more warmup only delays them
            for _ in range(5):
                nc.tensor.matmul(
                    wps[:1, :512], ones[:], wsrc[:], start=True, stop=True
                )

            pending_out = [None]  # deferred osb copy+DMA from the prior half
            pending_pv = [None]  # deferred last-PV closure from the prior half
            for s, e in enumerate(extents):
                # inputs on the sync queue only (outputs ride gpsimd) so
                # next-slot prefetch never queues behind output drains
                # first tile's K and first 512 q-cols land first so S(t0)
                # can start ~1.5us earlier at slot 0
                kt = kpool.tile([D, 16 * KT], BF16, tag="kt")
                nc.sync.dma_start(kt[:, :KT], kt_d[s][:, :KT])
                qt = qpool.tile([D, NQ], BF16, tag="qt")
                nc.sync.dma_start(qt[:, :QH], qt_d[s][:, :QH])
                bias = bpool.tile([KT, 16], F32, tag="bias")
                nc.sync.dma_start(bias[:, :e], b_d[s][:])
                if e > 1:
                    nc.sync.dma_start(kt[:, KT : e * KT], kt_d[s][:, KT:])
                nc.sync.dma_start(qt[:, QH:], qt_d[s][:, QH:])
                vt = vpool.tile([KT, 16 * KT], BF16, tag="v")
                nc.sync.dma_start(vt[:, : e * DV], v_d[s][:])

                g_slot = (e + 3) // 4
                for h in range(2):
                    q0 = h * QH
                    opsum = opool.tile([DV, QH], F32, tag="opsum")
                    pts = [None] * e

                    def den_group(lo, hi, gi, pts=pts, s=s, h=h, g_slot=g_slot):
                        """Sum P tiles lo..hi-1 on DVE (bf16 2x), DMA the
                        quad-sum out; the host reduces over k-partitions."""
                        n = hi - lo
                        if n == 1:
                            q = pts[lo]
                        elif n == 2:
                            q = pspool.tile([KT, QH], BF16, tag="ps")
                            nc.vector.tensor_tensor(
                                q[:], pts[lo][:], pts[lo + 1][:], mybir.AluOpType.add
                            )
                        else:
                            p1 = pspool.tile([KT, QH], BF16, tag="ps")
                            nc.vector.tensor_tensor(
                                p1[:], pts[lo][:], pts[lo + 1][:], mybir.AluOpType.add
                            )
                            if n == 3:
                                q = pqpool.tile([KT, QH], BF16, tag="pq")
                                nc.vector.tensor_tensor(
                                    q[:], p1[:], pts[lo + 2][:], mybir.AluOpType.add
                                )
                            else:
                                p2 = pspool.tile([KT, QH], BF16, tag="ps")
                                nc.vector.tensor_tensor(
                                    p2[:],
                                    pts[lo + 2][:],
                                    pts[lo + 3][:],
                                    mybir.AluOpType.add,
                                )
                                q = pqpool.tile([KT, QH], BF16, tag="pq")
                                nc.vector.tensor_tensor(
                                    q[:], p1[:], p2[:], mybir.AluOpType.add
                                )
                        nc.gpsimd.dma_start(d_d[s][h * g_slot + gi], q[:])

                    # software-pipelined: S-matmuls run one tile ahead of
                    # software-pipelined across slot-half boundaries: the
                    # previous half's LAST PV (+ its tail den group + output
                    # copy) is issued AFTER this half's S(t0)+exp(t0), so
                    # ScalarE never waits out a boundary.
                    def pv(t, opsum=opsum, vt=vt, pts=pts, e=e):
                        first, last = t == 0, t == e - 1
                        for c in range(2):
                            nc.tensor.matmul(
                                opsum[:, c * 512 : (c + 1) * 512],
                                vt[:, t * KT : (t + 1) * KT],
                                pts[t][:, c * 512 : (c + 1) * 512],
                                start=first,
                                stop=last,
                            )

                    def finish_half(
                        pv=pv, den_group=den_group, opsum=opsum, e=e, s=s, h=h, q0=q0
                    ):
                        t = e - 1
                        pv(t)
                        if t % 4 == 3:
                            den_group(t - 3, t + 1, t // 4)
                        else:
                            den_group(e - (e % 4), e, t // 4)
                        if s == S - 1 and h == 1:
                            # final tail: one copy + one DMA on the idle sync
                            # queue (the ~620ns trigger dominates the 400ns
                            # transfer, so chunking loses here)
                            osb = opool_sb.tile([DV, QH], BF16, tag="osb")
                            nc.vector.tensor_copy(osb[:], opsum[:])
                            nc.sync.dma_start(o_d[s][:, q0 : q0 + QH], osb[:])
                        else:

                            def pending(opsum=opsum, s=s, q0=q0):
                                osb = opool_sb.tile([DV, QH], BF16, tag="osb")
                                nc.vector.tensor_copy(osb[:], opsum[:])
                                nc.gpsimd.dma_start(
                                    o_d[s][:, q0 : q0 + QH], osb[:]
                                )

                            pending_out[0] = pending

                    for i in range(e):
                        t = i
                        spsum = spool.tile([KT, QH], F32)
                        for c in range(2):
                            nc.tensor.matmul(
                                spsum[:, c * 512 : (c + 1) * 512],
                                kt[:, t * KT : (t + 1) * KT],
                                qt[:, q0 + c * 512 : q0 + (c + 1) * 512],
                                start=True,
                                stop=True,
                            )
                        pt = ppool.tile([KT, QH], BF16)
                        nc.scalar.activation(
                            pt[:],
                            spsum[:],
                            mybir.ActivationFunctionType.Exp,
                            bias=bias[:, t : t + 1],
                            scale=SCALE,
                        )
                        pts[t] = pt
                        if i == 0:
                            # previous half's deferred last-PV + output copy
                            if pending_pv[0] is not None:
                                pending_pv[0]()
                                pending_pv[0] = None
                            if pending_out[0] is not None:
                                pending_out[0]()
                                pending_out[0] = None
                        if i > 0:
                            t = i - 1
                            pv(t)
                            # denominator: groups of 4 tiles quad-summed on
                            # DVE, shipped to HBM for host-side k-reduction
                            if t % 4 == 3:
                                den_group(t - 3, t + 1, t // 4)
                    pending_pv[0] = finish_half
                    if s == S - 1 and h == 1:
                        pending_pv[0]()
                        pending_pv[0] = None
    nc.compile()
    return nc


# ------------------------------------------------------------------- kernel
def kernel(queries, keys, values, valid_lens, _trace=False):
    global LAST_RESULT
    queries = np.asarray(queries, dtype=np.float32)
    keys = np.asarray(keys, dtype=np.float32)
    values = np.asarray(values, dtype=np.float32)
    valid_lens = np.asarray(valid_lens, dtype=np.int32)

    sizes = [int((int(l) + KT - 1) // KT) for l in valid_lens]
    extents, assign = _schedule(sizes)
    key = tuple(extents)
    if key not in _PROGRAM_CACHE:
        _PROGRAM_CACHE[key] = _build(extents)
    nc = _PROGRAM_CACHE[key]

    bf = ml_dtypes.bfloat16
    qT = np.ascontiguousarray(queries.transpose(0, 2, 1)).astype(bf)  # [B,D,NQ]
    kT = np.ascontiguousarray(keys.transpose(0, 2, 1)).astype(bf)  # [B,D,NK]
    v_bf = values.astype(bf)  # [B, NK, DV]
    # bias column per (batch, tile-row): 0 where key position valid else -1e6
    pos = np.arange(NK, dtype=np.int32).reshape(NK // KT, KT)  # [tiles, 128]
    bias_all = np.where(
        pos[None] < valid_lens[:, None, None], np.float32(0.0), NEG
    ).astype(np.float32)  # [B, tiles, 128]

    in_maps = []
    for c in range(NCORES):
        m = {}
        for s, e in enumerate(extents):
            seg = assign[c][s]
            qt = np.zeros((D, NQ), bf)
            kt = np.zeros((D, e * KT), bf)
            vv = np.zeros((KT, e * DV), bf)
            bb = np.full((KT, e), NEG, np.float32)
            if seg is not None:
                b, t0, n = seg
                qt[:] = qT[b]
                kt[:, : n * KT] = kT[b][:, t0 * KT : (t0 + n) * KT]
                # [n*KT, DV] -> SBUF image [KT, n*DV] (k-within-tile major)
                vv[:, : n * DV] = (
                    v_bf[b][t0 * KT : (t0 + n) * KT]
                    .reshape(n, KT, DV)
                    .transpose(1, 0, 2)
                    .reshape(KT, n * DV)
                )
                bb[:, :n] = bias_all[b][t0 : t0 + n].T
            m[f"qt{s}"] = qt
            m[f"kt{s}"] = kt
            m[f"v{s}"] = vv
            m[f"b{s}"] = bb
        in_maps.append(m)

    res = run_bass_kernel_spmd(
        nc, in_maps, core_ids=list(range(NCORES)), trace=_trace
    )
    LAST_RESULT = res

    o_acc = np.zeros((B, DV, NQ), np.float32)
    d_acc = np.zeros((B, NQ), np.float32)
    for c in range(NCORES):
        for s, e in enumerate(extents):
            seg = assign[c][s]
            if seg is None:
                continue
            b = seg[0]
            o_acc[b] += np.asarray(res.results[c][f"o{s}"], dtype=np.float32)
            g = (e + 3) // 4
            pq = np.asarray(res.results[c][f"pq{s}"], dtype=np.float32)
            d_acc[b] += pq.reshape(2, g * KT, QH).sum(axis=1).reshape(NQ)

    out = (o_acc / d_acc[:, None, :]).transpose(0, 2, 1)
    return np.ascontiguousarray(out.astype(np.float32))
